# revision 1
# baseline (speedup 1.0000x reference)
"""Trainium2 Bass kernel for CyclicShiftConv (Hilbert-rotation SE attention).

out[b,c,l] = sum_r softmax_r(MLP(mean_l x[b,c,rot_idx[r,l]]))[b,c,r] * x[b,c,rot_idx[r,l]]

Strategy (8 cores, data-parallel over batch; 2 samples = 512 (b,c) rows/core):
  - The gather indices rot_idx[r, l] are SHARED across all (b,c) rows, so the
    gather is done row-wise in transposed layout: X^T[l, bc] rows are 2 KiB
    contiguous chunks, gathered with the GPSIMD dma_gather engine.
  - mean over l of the gathered tensor == x @ (bincount(rot_idx[r])/L), so the
    SE-MLP input is a tiny matmul against a host-precomputed count vector; no
    gather needed before the MLP.
  - Gathered rows are returned to natural [bc, l] layout with PE transpose-mode
    matmuls (one PSUM tile per rotation); the softmax weights are applied and
    the 4 rotations summed by a scalar_tensor_tensor chain reading PSUM with
    per-partition weight scalars.
"""

import sys

for _p in ("/opt/trn_rl_repo", "/opt/pypackages"):
    if _p not in sys.path:
        sys.path.append(_p)

import numpy as np

B, C, L, R, RED = 16, 256, 4096, 4, 16
NCORES = 8
BPC = B // NCORES          # samples per core
BC = BPC * C               # 512 rows per core
NT = L // 128              # 32 l-tiles
M_CHUNK = 256              # l-positions per gather chunk
NM = L // M_CHUNK          # 16 chunks
GIDX = R * M_CHUNK         # 1024 gather indices per chunk (4 rotations concat)

# merged f32 constant block: [128, CST_W]
CST_ID = 0                 # identity [128, 128]
CST_CNT = 128              # cnt      [128, 128]
CST_W1 = 256               # w1       [128, 32]
CST_B2 = 288               # b2       [128, 2]
CST_W2 = 290               # w2       [16, 256] (partitions 0:16)
CST_B1 = 546               # b1       [16, 1]
CST_SEL = 547              # row-selector [4, 512]: sel[r, r*128:(r+1)*128]=1
CST_W = 1059

_NC_CACHE = {}


def _build_nc(loop_n=1):
    import concourse.bass as bass
    import concourse.mybir as mybir
    from concourse import bacc
    from concourse.tile import TileContext
    from contextlib import ExitStack

    f32 = mybir.dt.float32
    i16 = mybir.dt.int16
    AF = mybir.ActivationFunctionType
    AX = mybir.AxisListType
    ALU = mybir.AluOpType

    nc = bacc.Bacc(
        "TRN2",
        target_bir_lowering=False,
        debug=False,
        enable_asserts=False,
        num_devices=NCORES,
    )

    x_in = nc.dram_tensor("x", [BC, L], f32, kind="ExternalInput").ap()
    cst_in = nc.dram_tensor("cst", [128, CST_W], f32, kind="ExternalInput").ap()
    idx_in = nc.dram_tensor("idx", [128, NM * (GIDX // 16)], i16, kind="ExternalInput").ap()
    out = nc.dram_tensor("out", [BC, L], f32, kind="ExternalOutput").ap()

    with TileContext(nc) as tc, ExitStack() as ctx:
        if loop_n > 1:
            ctx.enter_context(tc.For_i(0, loop_n, 1))
        cpool = ctx.enter_context(tc.tile_pool(name="consts", bufs=1))
        dram = ctx.enter_context(tc.tile_pool(name="dram", bufs=1, space="DRAM"))
        xt_dram = [
            dram.tile([L, BC // 2], f32, name=f"xt_dram{h}") for h in range(2)
        ]

        cst = cpool.tile([128, CST_W], f32, name="cst")
        nc.sync.dma_start(cst[:], cst_in)
        idx_t = cpool.tile([128, NM * (GIDX // 16)], i16, name="idx_t")
        nc.sync.dma_start(idx_t[:], idx_in)

        ident = cst[:, CST_ID : CST_ID + 128]
        cnt_t = cst[:, CST_CNT : CST_CNT + 128]
        w1_t = cst[:, CST_W1 : CST_W1 + 32]
        b2_t = cst[:, CST_B2 : CST_B2 + 2]
        w2_t = cst[0:16, CST_W2 : CST_W2 + 256]
        b1_t = cst[0:16, CST_B1 : CST_B1 + 1]

        s_sb = cpool.tile([4, BC], f32, name="s_sb")
        wt_sb = cpool.tile([4, BC], f32, name="wt_sb")
        wb = [cpool.tile([128, BC], f32, name=f"wb{r}") for r in range(R)]

        # ---------------- phase 1: transpose x -> xt_dram, s = x @ cnt ----
        with (
            tc.tile_pool(name="xp", bufs=1) as xpool,
            tc.tile_pool(name="xtp", bufs=4) as xtpool,
            tc.tile_pool(name="pp1", bufs=6, space="PSUM") as pp1,
            tc.tile_pool(name="pps", bufs=1, space="PSUM") as pps,
        ):
            xs = []
            for j in range(4):
                xj = xpool.tile([128, L], f32, name=f"xs{j}")
                for h in range(4):
                    nc.sync.dma_start(
                        xj[:, h * 1024 : (h + 1) * 1024],
                        x_in[j * 128 : (j + 1) * 128, h * 1024 : (h + 1) * 1024],
                    )
                xs.append(xj)
            psum_s = pps.tile([4, BC], f32, name="psum_s")
            for half in range(2):
                for tq in range(NT // 4):
                    xt_t = xtpool.tile([128, 4, BC // 2], f32, name="xt_t")
                    for a in range(4):
                        t = 4 * tq + a
                        pt = pp1.tile([128, BC // 2], f32, name="pt")
                        for jj in range(2):
                            j = 2 * half + jj
                            nc.tensor.transpose(
                                pt[:, jj * 128 : (jj + 1) * 128],
                                xs[j][:, t * 128 : (t + 1) * 128],
                                ident,
                            )
                        nc.vector.tensor_copy(xt_t[:, a, :], pt[:])
                        nc.tensor.matmul(
                            psum_s[:, half * 256 : (half + 1) * 256],
                            cnt_t[:, 4 * t : 4 * t + 4],
                            xt_t[:, a, :],
                            start=(t == 0),
                            stop=(t == NT - 1),
                        )
                    for a in range(4):
                        t = 4 * tq + a
                        nc.scalar.dma_start(
                            xt_dram[half][t * 128 : (t + 1) * 128, :], xt_t[:, a, :]
                        )
            nc.vector.tensor_copy(s_sb[:], psum_s[:])

        # ---------------- SE MLP + softmax over rotations ------------------
        with (
            tc.tile_pool(name="mlp", bufs=1) as mpool,
            tc.tile_pool(name="ppm", bufs=1, space="PSUM") as ppm,
        ):
            sT = []
            for j in range(4):
                p_sT = ppm.tile([128, 4], f32, name="p_sT")
                nc.tensor.transpose(
                    p_sT[:], s_sb[:, j * 128 : (j + 1) * 128], cst[0:4, CST_ID : CST_ID + 4]
                )
                sTj = mpool.tile([128, 4], f32, name=f"sT{j}")
                nc.vector.tensor_copy(sTj[:], p_sT[:])
                sT.append(sTj)
            hs = []
            for b in range(BPC):
                p_h = ppm.tile([16, 4], f32, name="p_h")
                for hi in range(2):
                    nc.tensor.matmul(
                        p_h[:],
                        w1_t[:, hi * 16 : (hi + 1) * 16],
                        sT[2 * b + hi][:],
                        start=(hi == 0),
                        stop=(hi == 1),
                    )
                h_sb = mpool.tile([16, 4], f32, name=f"h{b}")
                nc.scalar.activation(h_sb[:], p_h[:], AF.Relu, bias=b1_t)
                hs.append(h_sb)
            p_sc = ppm.tile([128, 16], f32, name="p_sc")
            for b in range(BPC):
                for hi in range(2):
                    j = 2 * b + hi
                    nc.tensor.matmul(
                        p_sc[:, 4 * j : 4 * j + 4],
                        w2_t[:, hi * 128 : (hi + 1) * 128],
                        hs[b][:],
                        start=True, stop=True,
                    )
            sc_all = mpool.tile([128, 4, 4], f32, name="sc_all")
            b2ap = b2_t
            b2v = bass.AP(
                b2ap.tensor, b2ap.offset, [b2ap.ap[0], [0, 2], b2ap.ap[1], [0, 4]]
            )
            nc.vector.tensor_tensor(
                sc_all[:].rearrange("p (b hi) r -> p b hi r", b=2),
                p_sc[:].rearrange("p (b hi r) -> p b hi r", b=2, hi=2),
                b2v,
                op=ALU.add,
            )
            negmx = mpool.tile([128, 4], f32, name="negmx")
            nc.vector.reduce_max(negmx[:], sc_all[:], axis=AX.X, negate=True)
            nm = negmx[:]
            nmv = bass.AP(nm.tensor, nm.offset, [nm.ap[0], nm.ap[1], [0, 4]])
            nc.vector.tensor_tensor(sc_all[:], sc_all[:], nmv, op=ALU.add)
            e_all = mpool.tile([128, 4, 4], f32, name="e_all")
            nc.scalar.activation(
                e_all[:].rearrange("p a r -> p (a r)"),
                sc_all[:].rearrange("p a r -> p (a r)"),
                AF.Exp,
            )
            sm = mpool.tile([128, 4], f32, name="sm")
            nc.vector.reduce_sum(sm[:], e_all[:], axis=AX.X)
            rcp = mpool.tile([128, 4], f32, name="rcp")
            nc.vector.reciprocal(rcp[:], sm[:])
            rc = rcp[:]
            rcv = bass.AP(rc.tensor, rc.offset, [rc.ap[0], rc.ap[1], [0, 4]])
            W_all = mpool.tile([128, 4, 4], f32, name="W_all")
            nc.vector.tensor_tensor(W_all[:], e_all[:], rcv, op=ALU.mult)
            for j in range(4):
                p_wt = ppm.tile([4, 128], f32, name="p_wt")
                nc.tensor.transpose(p_wt[:], W_all[:, j, :], ident)
                nc.vector.tensor_copy(wt_sb[:, j * 128 : (j + 1) * 128], p_wt[:])
            for r in range(R):
                p_wb = ppm.tile([128, BC], f32, name="p_wb")
                nc.tensor.matmul(
                    p_wb[:],
                    cst[0:4, CST_SEL + r * 128 : CST_SEL + (r + 1) * 128],
                    wt_sb[:],
                    start=True, stop=True,
                )
                nc.vector.tensor_copy(wb[r][:], p_wb[:])

        # ---------------- phase 2: gather, transpose back, scale+sum ------
        with (
            tc.tile_pool(name="gp", bufs=5) as gpool,
            tc.tile_pool(name="op", bufs=1) as opool,
            tc.tile_pool(name="pp2", bufs=6, space="PSUM") as pp2,
        ):
            outs = [opool.tile([128, L], f32, name=f"os{j}") for j in range(4)]
            NGI = M_CHUNK // 128
            HB = BC // 2
            for m in range(NM):
                gh = []
                for half in range(2):
                    g = gpool.tile([128, NGI * R, HB], f32, name=f"g{half}")
                    nc.gpsimd.dma_gather(
                        g[:],
                        xt_dram[half][:],
                        idx_t[:, m * (GIDX // 16) : (m + 1) * (GIDX // 16)],
                        GIDX,
                        GIDX,
                        HB,
                    )
                    gh.append(g)
                for half in range(2):
                    g = gh[half]
                    for r in range(R):
                        gs = g[:, NGI * r : NGI * (r + 1), :]
                        wba = wb[r][:, half * HB : (half + 1) * HB]
                        wv = bass.AP(
                            wba.tensor, wba.offset, [wba.ap[0], [0, NGI], wba.ap[1]]
                        )
                        nc.vector.tensor_tensor(gs, gs, wv, op=ALU.mult)
                    for jj in range(2):
                        j = 2 * half + jj
                        po = pp2.tile([128, M_CHUNK], f32, name="po")
                        for gi in range(NGI):
                            for r in range(R):
                                nc.tensor.matmul(
                                    po[:, gi * 128 : (gi + 1) * 128],
                                    g[:, NGI * r + gi, jj * 128 : (jj + 1) * 128],
                                    ident,
                                    is_transpose=True,
                                    start=(r == 0),
                                    stop=(r == R - 1),
                                )
                        nc.scalar.copy(
                            outs[j][:, m * M_CHUNK : (m + 1) * M_CHUNK], po[:]
                        )
                if m % 4 == 3:
                    q = m // 4
                    W4 = 4 * M_CHUNK
                    for j in range(4):
                        nc.sync.dma_start(
                            out[j * 128 : (j + 1) * 128, q * W4 : (q + 1) * W4],
                            outs[j][:, q * W4 : (q + 1) * W4],
                        )

    nc.compile()
    return nc


def _host_prep(x, rot_idx, w1, b1, w2, b2):
    x = np.asarray(x, dtype=np.float32)
    rot_idx = np.asarray(rot_idx, dtype=np.int64)
    w1 = np.asarray(w1, dtype=np.float32)
    b1 = np.asarray(b1, dtype=np.float32)
    w2 = np.asarray(w2, dtype=np.float32)
    b2 = np.asarray(b2, dtype=np.float32)

    cnt = np.zeros((R, L), dtype=np.float32)
    for r in range(R):
        cnt[r] = np.bincount(rot_idx[r], minlength=L).astype(np.float32)
    cnt /= np.float32(L)
    # cnt_sb[p, 4t+r] = cnt[r, t*128+p]
    cnt_sb = np.ascontiguousarray(
        cnt.T.reshape(NT, 128, R).transpose(1, 0, 2).reshape(128, 128)
    )

    cst = np.zeros((128, CST_W), dtype=np.float32)
    cst[:, CST_ID : CST_ID + 128] = np.eye(128, dtype=np.float32)
    cst[:, CST_CNT : CST_CNT + 128] = cnt_sb
    cst[:, CST_W1 : CST_W1 + 32] = (
        w1.reshape(2, 128, RED).transpose(1, 0, 2).reshape(128, 2 * RED)
    )
    cst[:, CST_B2 : CST_B2 + 2] = b2.reshape(2, 128).T
    cst[0:16, CST_W2 : CST_W2 + 256] = w2
    cst[0:16, CST_B1] = b1
    for r in range(R):
        cst[r, CST_SEL + r * 128 : CST_SEL + (r + 1) * 128] = 1.0

    # gather index table: per chunk m, linear order [r0 l's..., r1 l's, ...],
    # wrapped idx_layout[p, s] = lin[s*16 + p], replicated over 8 core groups
    idx_sb = np.zeros((128, NM * (GIDX // 16)), dtype=np.int16)
    for m in range(NM):
        lin = np.concatenate(
            [rot_idx[r, m * M_CHUNK : (m + 1) * M_CHUNK] for r in range(R)]
        ).astype(np.int16)
        block = lin.reshape(GIDX // 16, 16).T  # [16, 64]
        idx_sb[:, m * (GIDX // 16) : (m + 1) * (GIDX // 16)] = np.tile(block, (8, 1))

    shared = {"cst": cst, "idx": idx_sb}
    in_maps = []
    for c in range(NCORES):
        m = dict(shared)
        m["x"] = np.ascontiguousarray(x[c * BPC : (c + 1) * BPC].reshape(BC, L))
        in_maps.append(m)
    return in_maps


def kernel(x, rot_idx, w1, b1, w2, b2, _trace=False):
    from concourse import bass_utils

    in_maps = _host_prep(x, rot_idx, w1, b1, w2, b2)
    if "nc" not in _NC_CACHE:
        _NC_CACHE["nc"] = _build_nc()
    nc = _NC_CACHE["nc"]
    res = bass_utils.run_bass_kernel_spmd(
        nc, in_maps, core_ids=list(range(NCORES)), trace=_trace
    )
    out = np.empty((B, C, L), dtype=np.float32)
    for c in range(NCORES):
        out[c * BPC : (c + 1) * BPC] = res.results[c]["out"].reshape(BPC, C, L)
    if _trace:
        kernel.last_results = res
    return out



# revision 7
# speedup vs baseline: 3.6661x; 3.6661x over previous
"""Trainium2 Bass kernel for CyclicShiftConv (Hilbert-rotation SE attention).

out[b,c,l] = sum_r softmax_r(MLP(mean_l x[b,c,rot_idx[r,l]]))[b,c,r] * x[b,c,rot_idx[r,l]]

Key mathematical facts exploited (verified at runtime in _derive_structure):
  1. Every rot_idx[r] is a PERMUTATION of [0, L).  Hence
     mean_l x[b,c,rot_idx[r,l]] is the same value for every r, so the MLP
     scores are identical across rotations and the softmax weights are
     exactly 1/4.  The whole SE-MLP collapses:
         out = 0.25 * (x + x_rot90 + x_rot180 + x_rot270).
  2. The Hilbert-curve rotation permutations have perfect block structure:
     every aligned 64-block of destination indices gathers from exactly one
     aligned 64-block of source indices, with only ~6 distinct intra-block
     patterns (12 distinct (pattern, 64-parity) pairs).  So each permutation
     is a PE matmul against a small set of constant one-hot routing matrices
     (entries 0.25 to fold in the softmax weight):
         psum[bc, j*64:(j+1)*64] += xT[s128-block]^T @ RM[pattern]
     This replaces the baseline's 32 MiB/core of DMA gather traffic with
     ~20us of Tensor-engine time.

Strategy (8 cores, data-parallel over batch; 2 samples = 512 (b,c) rows/core):
  - load x as bf16 (host converts; tolerance is 2e-2, bf16 adds ~2.4e-3)
  - PE-transpose x -> xT in SBUF (needed as matmul stationary)
  - 768 routing matmuls (64 moving cols each) accumulate the three rotated
    images, pre-scaled by 0.25, into PSUM
  - one fused scalar_tensor_tensor per (wave, bc-tile):
        out = (x * 0.25) + psum
  - DMA out as bf16; host upcasts to f32.
"""

import sys

for _p in ("/opt/trn_rl_repo", "/opt/pypackages"):
    if _p not in sys.path:
        sys.path.append(_p)

import numpy as np

B, C, L = 16, 256, 4096
R = 4
NCORES = 8
BPC = B // NCORES          # samples per core
BC = BPC * C               # 512 rows per core
NT4 = BC // 128            # 4 bc tiles
NB64 = L // 64             # 64 dst 64-blocks
NKILO = 4                  # 1024-col kilo blocks
NW = 8                     # 512-col dst waves
NKMAX = 12                 # max distinct routing matrices

_NC_CACHE = {}


def _derive_structure(rot_idx):
    """Derive the routing structure from the actual rot_idx at runtime."""
    rot = np.asarray(rot_idx, np.int64)
    assert rot.shape == (R, L)
    for r in range(R):
        assert np.array_equal(np.sort(rot[r]), np.arange(L)), (
            "rot_idx rows must be permutations (softmax-collapse precondition)")
    assert np.array_equal(rot[0], np.arange(L)), "rotation 0 must be identity"

    pats = {}
    KIDX = np.zeros((R, NB64), np.int64)
    SRC128 = np.zeros((R, NB64), np.int64)
    for r in (1, 2, 3):
        for j in range(NB64):
            src = rot[r, j * 64:(j + 1) * 64]
            m = int(src[0]) // 64
            assert np.all(src // 64 == m), "64-block structure violated"
            key = (tuple((src % 64).tolist()), m % 2)
            KIDX[r, j] = pats.setdefault(key, len(pats))
            SRC128[r, j] = m // 2
    assert len(pats) <= NKMAX, f"too many routing patterns: {len(pats)}"

    RM = np.zeros((128, NKMAX * 64), np.float32)
    for (pi, parity), k in pats.items():
        RM[np.asarray(pi, np.int64) + parity * 64, k * 64 + np.arange(64)] = 0.25

    # source kilo-blocks needed by each 512-col dst wave
    need = []
    for w in range(NW):
        js = range(w * 8, (w + 1) * 8)
        need.append({int(SRC128[r, j]) // 8 for r in (1, 2, 3) for j in js})
    # load last the kilo that the most waves can do without
    best = max(range(NKILO), key=lambda k: sum(k not in s for s in need))
    LO = [k for k in range(NKILO) if k != best] + [best]
    early = [w for w in range(NW) if best not in need[w]]
    late = [w for w in range(NW) if best in need[w]]
    return RM, KIDX, SRC128, LO, early, late


def _build_nc(KIDX, SRC128, LO, early, late):
    import concourse.mybir as mybir
    from concourse import bacc
    from concourse.tile import TileContext
    from contextlib import ExitStack

    f32 = mybir.dt.float32
    bf16 = mybir.dt.bfloat16
    ALU = mybir.AluOpType
    CSTW = NKMAX * 64 + 128

    nc = bacc.Bacc(
        "TRN2",
        target_bir_lowering=False,
        debug=False,
        enable_asserts=False,
        num_devices=NCORES,
    )

    x_in = nc.dram_tensor("x", [BC, L], bf16, kind="ExternalInput").ap()
    cst_in = nc.dram_tensor("cst", [128, CSTW], bf16, kind="ExternalInput").ap()
    out = nc.dram_tensor("out", [BC, L], bf16, kind="ExternalOutput").ap()

    with TileContext(nc) as tc, ExitStack() as ctx:
        cpool = ctx.enter_context(tc.tile_pool(name="consts", bufs=1))
        xpool = ctx.enter_context(tc.tile_pool(name="xs", bufs=1))
        tpool = ctx.enter_context(tc.tile_pool(name="xT", bufs=1))
        opool = ctx.enter_context(tc.tile_pool(name="ostage", bufs=1))

        cst = cpool.tile([128, CSTW], bf16, name="cst")
        nc.sync.dma_start(cst[:], cst_in)
        rm = cst[:, 0:NKMAX * 64]
        ident = cst[:, NKMAX * 64:NKMAX * 64 + 128]

        xs = [xpool.tile([128, L], bf16, name=f"xs{t}") for t in range(NT4)]
        xT = [tpool.tile([128, 8, BC], bf16, name=f"xT{k}") for k in range(NKILO)]
        ost = [opool.tile([128, L], bf16, name=f"os{t}") for t in range(NT4)]
        gpool = ctx.enter_context(tc.tile_pool(name="gtmp", bufs=3))

        # all input loads up front, in kilo load-order (transfers serialize
        # on the DMA engines in issue order)
        for k in LO:
            for t in range(NT4):
                nc.sync.dma_start(
                    xs[t][:, k * 1024:(k + 1) * 1024],
                    x_in[t * 128:(t + 1) * 128, k * 1024:(k + 1) * 1024],
                )

        state = {"ev": 0, "fin": 0}

        with (
            tc.tile_pool(name="pb", bufs=2, space="PSUM") as pbpool,
            tc.tile_pool(name="pc", bufs=4, space="PSUM") as pcpool,
        ):
            def do_kilo(k):
                # transpose kilo k of x into xT[k]
                for e in range(8):
                    lb = k * 8 + e
                    pb = pbpool.tile([128, BC], bf16, name="pb")
                    for t in range(NT4):
                        nc.tensor.transpose(
                            pb[:, t * 128:(t + 1) * 128],
                            xs[t][:, lb * 128:(lb + 1) * 128],
                            ident,
                        )
                    i = state["ev"] % 2
                    state["ev"] += 1
                    if i == 0:
                        nc.vector.tensor_copy(xT[k][:, e, :], pb[:])
                    else:
                        nc.scalar.copy(xT[k][:, e, :], pb[:])

            def do_wave(w):
                for t in range(NT4):
                    pc = pcpool.tile([128, 512], f32, name="pc")
                    for jj in range(8):
                        j = w * 8 + jj
                        for ri, r in enumerate((1, 2, 3)):
                            s = int(SRC128[r, j])
                            kk = int(KIDX[r, j])
                            nc.tensor.matmul(
                                pc[:, jj * 64:(jj + 1) * 64],
                                xT[s // 8][:, s % 8, t * 128:(t + 1) * 128],
                                rm[:, kk * 64:(kk + 1) * 64],
                                start=(ri == 0),
                                stop=(ri == 2),
                            )
                    # GPSIMD cannot access PSUM (or run TensorScalarPtr):
                    # alternate a DVE-solo fused final with an ACT psum->SBUF
                    # copy followed by a cheap all-SBUF DVE stt
                    if state["fin"] % 2 == 0:
                        nc.vector.scalar_tensor_tensor(
                            ost[t][:, w * 512:(w + 1) * 512],
                            xs[t][:, w * 512:(w + 1) * 512],
                            0.25,
                            pc[:],
                            ALU.mult,
                            ALU.add,
                        )
                    else:
                        gt = gpool.tile([128, 512], bf16, name="gt")
                        nc.scalar.copy(gt[:], pc[:])
                        nc.vector.scalar_tensor_tensor(
                            ost[t][:, w * 512:(w + 1) * 512],
                            xs[t][:, w * 512:(w + 1) * 512],
                            0.25,
                            gt[:],
                            ALU.mult,
                            ALU.add,
                        )
                    state["fin"] += 1

            for k in LO[:3]:
                do_kilo(k)
            for w in early:
                do_wave(w)
            do_kilo(LO[3])
            for w in late:
                do_wave(w)

            # output DMA in half-row chunks per bc-tile, issued from SP
            # (idle after the input loads); waves cover dst columns in
            # 512-col slices, chunk h covers waves with w*512 in the half
            for t in range(NT4):
                for h in range(2):
                    nc.sync.dma_start(
                        out[t * 128:(t + 1) * 128, h * 2048:(h + 1) * 2048],
                        ost[t][:, h * 2048:(h + 1) * 2048],
                    )

    nc.compile()
    return nc


def _host_prep(x, rot_idx):
    import ml_dtypes

    bf = ml_dtypes.bfloat16
    RM = _NC_CACHE["RM"]
    cst = np.zeros((128, NKMAX * 64 + 128), np.float32)
    cst[:, :NKMAX * 64] = RM
    cst[:, NKMAX * 64:] = np.eye(128, dtype=np.float32)
    cst = cst.astype(bf)

    x = np.asarray(x, dtype=np.float32)
    in_maps = []
    for c in range(NCORES):
        xc = np.ascontiguousarray(
            x[c * BPC:(c + 1) * BPC].reshape(BC, L)).astype(bf)
        in_maps.append({"x": xc, "cst": cst})
    return in_maps


def kernel(x, rot_idx, w1, b1, w2, b2, _trace=False):
    # w1/b1/w2/b2 provably do not affect the output when every rot_idx row
    # is a permutation (asserted in _derive_structure): the SE-MLP sees the
    # same mean for every rotation, so the softmax is uniform.
    from concourse import bass_utils

    key = np.asarray(rot_idx, np.int32).tobytes()
    if _NC_CACHE.get("key") != key:
        RM, KIDX, SRC128, LO, early, late = _derive_structure(rot_idx)
        _NC_CACHE["RM"] = RM
        _NC_CACHE["nc"] = _build_nc(KIDX, SRC128, LO, early, late)
        _NC_CACHE["key"] = key
    nc = _NC_CACHE["nc"]

    in_maps = _host_prep(x, rot_idx)
    res = bass_utils.run_bass_kernel_spmd(
        nc, in_maps, core_ids=list(range(NCORES)), trace=_trace
    )
    out = np.empty((B, C, L), dtype=np.float32)
    for c in range(NCORES):
        out[c * BPC:(c + 1) * BPC] = (
            res.results[c]["out"].astype(np.float32).reshape(BPC, C, L))
    if _trace:
        kernel.last_results = res
    return out


# revision 11
# speedup vs baseline: 4.1085x; 1.1207x over previous
"""Trainium2 Bass kernel for CyclicShiftConv (Hilbert-rotation SE attention).

out[b,c,l] = sum_r softmax_r(MLP(mean_l x[b,c,rot_idx[r,l]]))[b,c,r] * x[b,c,rot_idx[r,l]]

Key mathematical facts exploited (verified at runtime in _derive_structure):
  1. Every rot_idx[r] is a PERMUTATION of [0, L).  Hence
     mean_l x[b,c,rot_idx[r,l]] is the same value for every r, so the MLP
     scores are identical across rotations and the softmax weights are
     exactly 1/4.  The whole SE-MLP collapses:
         out = 0.25 * (x + x_rot90 + x_rot180 + x_rot270).
  2. The Hilbert-curve rotation permutations have perfect block structure:
     every aligned 64-block of destination indices gathers from exactly one
     aligned 64-block of source indices, with only ~6 distinct intra-block
     patterns (12 distinct (pattern, 64-parity) pairs).  So each permutation
     is a PE matmul against a small set of constant one-hot routing matrices
     (entries 0.25 to fold in the softmax weight):
         psum[bc, j*64:(j+1)*64] += xT[s128-block]^T @ RM[pattern]
     This replaces the baseline's 32 MiB/core of DMA gather traffic with
     ~20us of Tensor-engine time.

Strategy (8 cores, data-parallel over batch; 2 samples = 512 (b,c) rows/core):
  - load x as bf16 (host converts; tolerance is 2e-2, bf16 adds ~2.4e-3)
  - PE-transpose x -> xT in SBUF (needed as matmul stationary)
  - 768 routing matmuls (64 moving cols each) accumulate the three rotated
    images, pre-scaled by 0.25, into PSUM
  - one fused scalar_tensor_tensor per (wave, bc-tile):
        out = (x * 0.25) + psum
  - DMA out as bf16; host upcasts to f32.
"""

import sys

for _p in ("/opt/trn_rl_repo", "/opt/pypackages"):
    if _p not in sys.path:
        sys.path.append(_p)

import numpy as np

B, C, L = 16, 256, 4096
R = 4
NCORES = 8
BPC = B // NCORES          # samples per core
BC = BPC * C               # 512 rows per core
NT4 = BC // 128            # 4 bc tiles
NB64 = L // 64             # 64 dst 64-blocks
NKILO = 4                  # 1024-col kilo blocks
NW = 8                     # 512-col dst waves
NKMAX = 12                 # max distinct routing matrices

_NC_CACHE = {}


def _derive_structure(rot_idx):
    """Derive the routing structure from the actual rot_idx at runtime."""
    rot = np.asarray(rot_idx, np.int64)
    assert rot.shape == (R, L)
    for r in range(R):
        assert np.array_equal(np.sort(rot[r]), np.arange(L)), (
            "rot_idx rows must be permutations (softmax-collapse precondition)")
    assert np.array_equal(rot[0], np.arange(L)), "rotation 0 must be identity"

    pats = {}
    KIDX = np.zeros((R, NB64), np.int64)
    SRC128 = np.zeros((R, NB64), np.int64)
    for r in (1, 2, 3):
        for j in range(NB64):
            src = rot[r, j * 64:(j + 1) * 64]
            m = int(src[0]) // 64
            assert np.all(src // 64 == m), "64-block structure violated"
            key = (tuple((src % 64).tolist()), m % 2)
            KIDX[r, j] = pats.setdefault(key, len(pats))
            SRC128[r, j] = m // 2
    assert len(pats) <= NKMAX, f"too many routing patterns: {len(pats)}"

    RM = np.zeros((128, NKMAX * 64), np.float32)
    for (pi, parity), k in pats.items():
        RM[np.asarray(pi, np.int64) + parity * 64, k * 64 + np.arange(64)] = 0.25

    # source kilo-blocks needed by each 1024-col dst kilo-wave
    need = []
    for w in range(NKILO):
        js = range(w * 16, (w + 1) * 16)
        need.append({int(SRC128[r, j]) // 8 for r in (1, 2, 3) for j in js})
    # load last the kilo that the most waves can do without
    best = max(range(NKILO), key=lambda k: sum(k not in s for s in need))
    LO = [k for k in range(NKILO) if k != best] + [best]
    early = [w for w in range(NKILO) if best not in need[w]]
    late = [w for w in range(NKILO) if best in need[w]]
    return RM, KIDX, SRC128, LO, early, late


def _build_nc(KIDX, SRC128, LO, early, late):
    import concourse.mybir as mybir
    from concourse import bacc
    from concourse.tile import TileContext
    from contextlib import ExitStack

    f32 = mybir.dt.float32
    bf16 = mybir.dt.bfloat16
    ALU = mybir.AluOpType
    CSTW = NKMAX * 64 + 128

    nc = bacc.Bacc(
        "TRN2",
        target_bir_lowering=False,
        debug=False,
        enable_asserts=False,
        num_devices=NCORES,
    )

    x_in = nc.dram_tensor("x", [BC, L], bf16, kind="ExternalInput").ap()
    cst_in = nc.dram_tensor("cst", [128, CSTW], bf16, kind="ExternalInput").ap()
    out = nc.dram_tensor("out", [BC, L], bf16, kind="ExternalOutput").ap()

    with TileContext(nc) as tc, ExitStack() as ctx:
        cpool = ctx.enter_context(tc.tile_pool(name="consts", bufs=1))
        xpool = ctx.enter_context(tc.tile_pool(name="xs", bufs=1))
        tpool = ctx.enter_context(tc.tile_pool(name="xT", bufs=1))
        opool = ctx.enter_context(tc.tile_pool(name="ostage", bufs=1))

        cst = cpool.tile([128, CSTW], bf16, name="cst")
        nc.sync.dma_start(cst[:], cst_in)
        rm = cst[:, 0:NKMAX * 64]
        ident = cst[:, NKMAX * 64:NKMAX * 64 + 128]

        xs = [xpool.tile([128, L], bf16, name=f"xs{t}") for t in range(NT4)]
        xT = [tpool.tile([128, 8, BC], bf16, name=f"xT{k}") for k in range(NKILO)]
        ost = [opool.tile([128, L], bf16, name=f"os{t}") for t in range(NT4)]
        gpool = ctx.enter_context(tc.tile_pool(name="gtmp", bufs=3))

        # all input loads up front, in kilo load-order (transfers serialize
        # on the DMA engines in issue order)
        for k in LO:
            for t in range(NT4):
                nc.sync.dma_start(
                    xs[t][:, k * 1024:(k + 1) * 1024],
                    x_in[t * 128:(t + 1) * 128, k * 1024:(k + 1) * 1024],
                )

        # xq = 0.25 * x, precomputed with cheap 4x-mode tensor_scalar ops on
        # the otherwise-idle DVE during the load phase; finals then become
        # ost = xq + psum (tensor_tensor add) or a fused stt
        xq = [opool.tile([128, L], bf16, name=f"xq{t}") for t in range(NT4)]
        for k in LO:
            for t in range(NT4):
                nc.vector.tensor_scalar_mul(
                    xq[t][:, k * 1024:(k + 1) * 1024],
                    xs[t][:, k * 1024:(k + 1) * 1024], 0.25)

        state = {"ev": 0, "fin": 0}

        with (
            tc.tile_pool(name="pb", bufs=4, space="PSUM") as pbpool,
            tc.tile_pool(name="pc", bufs=2, space="PSUM") as pcpool,
        ):
            def do_kilo(k):
                # transpose kilo k of x into xT[k]
                for e in range(8):
                    lb = k * 8 + e
                    pb = pbpool.tile([128, BC], bf16, name="pb")
                    for t in range(NT4):
                        nc.tensor.transpose(
                            pb[:, t * 128:(t + 1) * 128],
                            xs[t][:, lb * 128:(lb + 1) * 128],
                            ident,
                        )
                    i = state["ev"] % 2
                    state["ev"] += 1
                    if i == 0:
                        nc.vector.tensor_copy(xT[k][:, e, :], pb[:])
                    else:
                        nc.scalar.copy(xT[k][:, e, :], pb[:])

            def do_wave(w):
                # w is a 1024-col dst kilo-wave
                for t in range(NT4):
                    pc = pcpool.tile([128, 1024], f32, name="pc")
                    for jj in range(16):
                        j = w * 16 + jj
                        for ri, r in enumerate((1, 2, 3)):
                            s = int(SRC128[r, j])
                            kk = int(KIDX[r, j])
                            nc.tensor.matmul(
                                pc[:, jj * 64:(jj + 1) * 64],
                                xT[s // 8][:, s % 8, t * 128:(t + 1) * 128],
                                rm[:, kk * 64:(kk + 1) * 64],
                                start=(ri == 0),
                                stop=(ri == 2),
                            )
                    osl = ost[t][:, w * 1024:(w + 1) * 1024]
                    xql = xq[t][:, w * 1024:(w + 1) * 1024]
                    # GPSIMD cannot access PSUM (or run TensorScalarPtr):
                    # rotate DVE-solo stt | ACT evict + Pool tt-add |
                    # ACT evict + DVE tt-add
                    m = state["fin"] % 4
                    if m in (0, 2):
                        nc.vector.scalar_tensor_tensor(
                            osl, xs[t][:, w * 1024:(w + 1) * 1024],
                            0.25, pc[:], ALU.mult, ALU.add)
                    else:
                        gt = gpool.tile([128, 1024], bf16, name="gt")
                        nc.scalar.copy(gt[:], pc[:])
                        if m == 1:
                            nc.gpsimd.tensor_tensor(osl, xql, gt[:], op=ALU.add)
                        else:
                            nc.vector.tensor_tensor(osl, xql, gt[:], op=ALU.add)
                    state["fin"] += 1
                    # stream the result out immediately (SP is idle by now)
                    nc.sync.dma_start(
                        out[t * 128:(t + 1) * 128, w * 1024:(w + 1) * 1024],
                        osl,
                    )

            for k in LO[:3]:
                do_kilo(k)
            for w in early:
                do_wave(w)
            do_kilo(LO[3])
            for w in late:
                do_wave(w)

    nc.compile()
    return nc


def _host_prep(x, rot_idx):
    import ml_dtypes

    bf = ml_dtypes.bfloat16
    RM = _NC_CACHE["RM"]
    cst = np.zeros((128, NKMAX * 64 + 128), np.float32)
    cst[:, :NKMAX * 64] = RM
    cst[:, NKMAX * 64:] = np.eye(128, dtype=np.float32)
    cst = cst.astype(bf)

    x = np.asarray(x, dtype=np.float32)
    in_maps = []
    for c in range(NCORES):
        xc = np.ascontiguousarray(
            x[c * BPC:(c + 1) * BPC].reshape(BC, L)).astype(bf)
        in_maps.append({"x": xc, "cst": cst})
    return in_maps


def kernel(x, rot_idx, w1, b1, w2, b2, _trace=False):
    # w1/b1/w2/b2 provably do not affect the output when every rot_idx row
    # is a permutation (asserted in _derive_structure): the SE-MLP sees the
    # same mean for every rotation, so the softmax is uniform.
    from concourse import bass_utils

    key = np.asarray(rot_idx, np.int32).tobytes()
    if _NC_CACHE.get("key") != key:
        RM, KIDX, SRC128, LO, early, late = _derive_structure(rot_idx)
        _NC_CACHE["RM"] = RM
        _NC_CACHE["nc"] = _build_nc(KIDX, SRC128, LO, early, late)
        _NC_CACHE["key"] = key
    nc = _NC_CACHE["nc"]

    in_maps = _host_prep(x, rot_idx)
    res = bass_utils.run_bass_kernel_spmd(
        nc, in_maps, core_ids=list(range(NCORES)), trace=_trace
    )
    out = np.empty((B, C, L), dtype=np.float32)
    for c in range(NCORES):
        out[c * BPC:(c + 1) * BPC] = (
            res.results[c]["out"].astype(np.float32).reshape(BPC, C, L))
    if _trace:
        kernel.last_results = res
    return out


# revision 12
# speedup vs baseline: 4.1740x; 1.0159x over previous
"""Trainium2 Bass kernel for CyclicShiftConv (Hilbert-rotation SE attention).

out[b,c,l] = sum_r softmax_r(MLP(mean_l x[b,c,rot_idx[r,l]]))[b,c,r] * x[b,c,rot_idx[r,l]]

Key mathematical facts exploited (verified at runtime in _derive_structure):
  1. Every rot_idx[r] is a PERMUTATION of [0, L).  Hence
     mean_l x[b,c,rot_idx[r,l]] is the same value for every r, so the MLP
     scores are identical across rotations and the softmax weights are
     exactly 1/4.  The whole SE-MLP collapses:
         out = 0.25 * (x + x_rot90 + x_rot180 + x_rot270).
  2. The Hilbert-curve rotation permutations have perfect block structure:
     every aligned 64-block of destination indices gathers from exactly one
     aligned 64-block of source indices, with only ~6 distinct intra-block
     patterns (12 distinct (pattern, 64-parity) pairs).  So each permutation
     is a PE matmul against a small set of constant one-hot routing matrices
     (entries 0.25 to fold in the softmax weight):
         psum[bc, j*64:(j+1)*64] += xT[s128-block]^T @ RM[pattern]
     This replaces the baseline's 32 MiB/core of DMA gather traffic with
     ~20us of Tensor-engine time.

Strategy (8 cores, data-parallel over batch; 2 samples = 512 (b,c) rows/core):
  - load x as bf16 (host converts; tolerance is 2e-2, bf16 adds ~2.4e-3)
  - PE-transpose x -> xT in SBUF (needed as matmul stationary)
  - 768 routing matmuls (64 moving cols each) accumulate the three rotated
    images, pre-scaled by 0.25, into PSUM
  - one fused scalar_tensor_tensor per (wave, bc-tile):
        out = (x * 0.25) + psum
  - DMA out as bf16; host upcasts to f32.
"""

import sys

for _p in ("/opt/trn_rl_repo", "/opt/pypackages"):
    if _p not in sys.path:
        sys.path.append(_p)

import numpy as np

B, C, L = 16, 256, 4096
R = 4
NCORES = 8
BPC = B // NCORES          # samples per core
BC = BPC * C               # 512 rows per core
NT4 = BC // 128            # 4 bc tiles
NB64 = L // 64             # 64 dst 64-blocks
NKILO = 4                  # 1024-col kilo blocks
NW = 8                     # 512-col dst waves
NKMAX = 12                 # max distinct routing matrices

_NC_CACHE = {}


def _derive_structure(rot_idx):
    """Derive the routing structure from the actual rot_idx at runtime."""
    rot = np.asarray(rot_idx, np.int64)
    assert rot.shape == (R, L)
    for r in range(R):
        assert np.array_equal(np.sort(rot[r]), np.arange(L)), (
            "rot_idx rows must be permutations (softmax-collapse precondition)")
    assert np.array_equal(rot[0], np.arange(L)), "rotation 0 must be identity"

    pats = {}
    KIDX = np.zeros((R, NB64), np.int64)
    SRC128 = np.zeros((R, NB64), np.int64)
    for r in (1, 2, 3):
        for j in range(NB64):
            src = rot[r, j * 64:(j + 1) * 64]
            m = int(src[0]) // 64
            assert np.all(src // 64 == m), "64-block structure violated"
            key = (tuple((src % 64).tolist()), m % 2)
            KIDX[r, j] = pats.setdefault(key, len(pats))
            SRC128[r, j] = m // 2
    assert len(pats) <= NKMAX, f"too many routing patterns: {len(pats)}"

    RM = np.zeros((128, NKMAX * 64), np.float32)
    for (pi, parity), k in pats.items():
        RM[np.asarray(pi, np.int64) + parity * 64, k * 64 + np.arange(64)] = 0.25

    # source kilo-blocks needed by each 1024-col dst kilo-wave
    need = []
    for w in range(NKILO):
        js = range(w * 16, (w + 1) * 16)
        need.append({int(SRC128[r, j]) // 8 for r in (1, 2, 3) for j in js})
    # load last the kilo that the most waves can do without
    best = max(range(NKILO), key=lambda k: sum(k not in s for s in need))
    LO = [k for k in range(NKILO) if k != best] + [best]
    early = [w for w in range(NKILO) if best not in need[w]]
    late = [w for w in range(NKILO) if best in need[w]]
    return RM, KIDX, SRC128, LO, early, late


def _build_nc(KIDX, SRC128, LO, early, late):
    import concourse.mybir as mybir
    from concourse import bacc
    from concourse.tile import TileContext
    from contextlib import ExitStack

    f32 = mybir.dt.float32
    bf16 = mybir.dt.bfloat16
    ALU = mybir.AluOpType
    CSTW = NKMAX * 64 + 128

    nc = bacc.Bacc(
        "TRN2",
        target_bir_lowering=False,
        debug=False,
        enable_asserts=False,
        num_devices=NCORES,
    )

    x_in = nc.dram_tensor("x", [BC, L], bf16, kind="ExternalInput").ap()
    cst_in = nc.dram_tensor("cst", [128, CSTW], bf16, kind="ExternalInput").ap()
    out = nc.dram_tensor("out", [BC, L], bf16, kind="ExternalOutput").ap()

    with TileContext(nc) as tc, ExitStack() as ctx:
        cpool = ctx.enter_context(tc.tile_pool(name="consts", bufs=1))
        xpool = ctx.enter_context(tc.tile_pool(name="xs", bufs=1))
        tpool = ctx.enter_context(tc.tile_pool(name="xT", bufs=1))
        opool = ctx.enter_context(tc.tile_pool(name="ostage", bufs=1))

        cst = cpool.tile([128, CSTW], bf16, name="cst")
        nc.sync.dma_start(cst[:], cst_in)
        rm = cst[:, 0:NKMAX * 64]
        ident = cst[:, NKMAX * 64:NKMAX * 64 + 128]

        xs = [xpool.tile([128, L], bf16, name=f"xs{t}") for t in range(NT4)]
        xT = [tpool.tile([128, 8, BC], bf16, name=f"xT{k}") for k in range(NKILO)]
        ost = [opool.tile([128, L], bf16, name=f"os{t}") for t in range(NT4)]
        gpool = ctx.enter_context(tc.tile_pool(name="gtmp", bufs=3))

        # all input loads up front, in kilo load-order (transfers serialize
        # on the DMA engines in issue order)
        for k in LO:
            for t in range(NT4):
                nc.sync.dma_start(
                    xs[t][:, k * 1024:(k + 1) * 1024],
                    x_in[t * 128:(t + 1) * 128, k * 1024:(k + 1) * 1024],
                )

        # xq = 0.25 * x, precomputed with cheap 4x-mode tensor_scalar ops on
        # the otherwise-idle DVE during the load phase; finals then become
        # ost = xq + psum (tensor_tensor add) or a fused stt
        xq = [opool.tile([128, L], bf16, name=f"xq{t}") for t in range(NT4)]
        for k in LO:
            for t in range(NT4):
                nc.vector.tensor_scalar_mul(
                    xq[t][:, k * 1024:(k + 1) * 1024],
                    xs[t][:, k * 1024:(k + 1) * 1024], 0.25)

        state = {"ev": 0, "fin": 0}

        with (
            tc.tile_pool(name="pb", bufs=2, space="PSUM") as pbpool,
            tc.tile_pool(name="pc", bufs=3, space="PSUM") as pcpool,
        ):
            def do_kilo(k):
                # transpose kilo k of x into xT[k], two l-blocks per psum
                # tile ([128,1024] bf16 = one 2KB bank), evicted in one op
                for e2 in range(4):
                    pb = pbpool.tile([128, 2, BC], bf16, name="pb")
                    for eh in range(2):
                        lb = k * 8 + e2 * 2 + eh
                        for t in range(NT4):
                            nc.tensor.transpose(
                                pb[:, eh, t * 128:(t + 1) * 128],
                                xs[t][:, lb * 128:(lb + 1) * 128],
                                ident,
                            )
                    # DVE evicts run in 2x mode (392ns/512 cols); ACT takes
                    # every third one to keep DVE free for xq work
                    i = state["ev"] % 3
                    state["ev"] += 1
                    dst = xT[k][:, e2 * 2:e2 * 2 + 2, :]
                    if i == 2:
                        nc.scalar.copy(dst, pb[:])
                    else:
                        nc.vector.tensor_copy(dst, pb[:])

            def do_wave(w):
                # w is a 1024-col dst kilo-wave
                for t in range(NT4):
                    pc = pcpool.tile([128, 1024], f32, name="pc")
                    for jj in range(16):
                        j = w * 16 + jj
                        for ri, r in enumerate((1, 2, 3)):
                            s = int(SRC128[r, j])
                            kk = int(KIDX[r, j])
                            nc.tensor.matmul(
                                pc[:, jj * 64:(jj + 1) * 64],
                                xT[s // 8][:, s % 8, t * 128:(t + 1) * 128],
                                rm[:, kk * 64:(kk + 1) * 64],
                                start=(ri == 0),
                                stop=(ri == 2),
                            )
                    osl = ost[t][:, w * 1024:(w + 1) * 1024]
                    xql = xq[t][:, w * 1024:(w + 1) * 1024]
                    # GPSIMD cannot access PSUM (or run TensorScalarPtr):
                    # rotate DVE-solo stt | ACT evict + Pool tt-add |
                    # ACT evict + DVE tt-add
                    m = state["fin"] % 4
                    if m in (0, 2):
                        nc.vector.scalar_tensor_tensor(
                            osl, xs[t][:, w * 1024:(w + 1) * 1024],
                            0.25, pc[:], ALU.mult, ALU.add)
                    else:
                        gt = gpool.tile([128, 1024], bf16, name="gt")
                        nc.scalar.copy(gt[:], pc[:])
                        if m == 1:
                            nc.gpsimd.tensor_tensor(osl, xql, gt[:], op=ALU.add)
                        else:
                            nc.vector.tensor_tensor(osl, xql, gt[:], op=ALU.add)
                    state["fin"] += 1
                    # stream the result out immediately (SP is idle by now)
                    nc.sync.dma_start(
                        out[t * 128:(t + 1) * 128, w * 1024:(w + 1) * 1024],
                        osl,
                    )

            for k in LO[:3]:
                do_kilo(k)
            for w in early:
                do_wave(w)
            do_kilo(LO[3])
            for w in late:
                do_wave(w)

    nc.compile()
    return nc


def _host_prep(x, rot_idx):
    import ml_dtypes

    bf = ml_dtypes.bfloat16
    RM = _NC_CACHE["RM"]
    cst = np.zeros((128, NKMAX * 64 + 128), np.float32)
    cst[:, :NKMAX * 64] = RM
    cst[:, NKMAX * 64:] = np.eye(128, dtype=np.float32)
    cst = cst.astype(bf)

    x = np.asarray(x, dtype=np.float32)
    in_maps = []
    for c in range(NCORES):
        xc = np.ascontiguousarray(
            x[c * BPC:(c + 1) * BPC].reshape(BC, L)).astype(bf)
        in_maps.append({"x": xc, "cst": cst})
    return in_maps


def kernel(x, rot_idx, w1, b1, w2, b2, _trace=False):
    # w1/b1/w2/b2 provably do not affect the output when every rot_idx row
    # is a permutation (asserted in _derive_structure): the SE-MLP sees the
    # same mean for every rotation, so the softmax is uniform.
    from concourse import bass_utils

    key = np.asarray(rot_idx, np.int32).tobytes()
    if _NC_CACHE.get("key") != key:
        RM, KIDX, SRC128, LO, early, late = _derive_structure(rot_idx)
        _NC_CACHE["RM"] = RM
        _NC_CACHE["nc"] = _build_nc(KIDX, SRC128, LO, early, late)
        _NC_CACHE["key"] = key
    nc = _NC_CACHE["nc"]

    in_maps = _host_prep(x, rot_idx)
    res = bass_utils.run_bass_kernel_spmd(
        nc, in_maps, core_ids=list(range(NCORES)), trace=_trace
    )
    out = np.empty((B, C, L), dtype=np.float32)
    for c in range(NCORES):
        out[c * BPC:(c + 1) * BPC] = (
            res.results[c]["out"].astype(np.float32).reshape(BPC, C, L))
    if _trace:
        kernel.last_results = res
    return out
